# revision 49
# baseline (speedup 1.0000x reference)
"""Trainium2 Bass kernel for AttentionLayerWithMask (ragged prefix-mask attention).

Problem: B=1024, S=200, D=O=512.
  sqlen = mask.sum(1); query = proj_q(x[b, sqlen-1]); keys/values = x[b, :sqlen-1]
  out = tanh(attn @ V)

Algebraic rewrite (exact, up to fp reassociation):
  scores[b,s] = (Wk^T q[b]) . x[b,s]  (+ const, softmax-invariant)
  out[b]      = tanh(Wv (sum_s attn[b,s] x[b,s]) + bv)

Ragged cell packing: valid key positions (avg ~102 of 200 per batch) are
packed into (lane, chunk) cells of CS positions, each cell holding tokens of
exactly one batch. A core processes 128 lanes x NC chunks (NC ~ 7 from the
actual mask, vs 8x25=200 position-slots in the dense layout). Per-chunk
one-hot routing matrices Wc tie lanes to batches: qk rows are gathered
per-cell on the TensorEngine before the score pass, and the softmax-weighted
context sum is formed by PE scatter-matmuls with pc-weighted one-hot weights
(ps_ctx += (Wc*pc_si)^T @ x_si; ps_z += (Wc*pc_si)^T @ ones), accumulated in
PSUM across all positions. The host folds per-batch score maxes m_b into the
additive mask, so pc = exp(score - m_b) lands in (0, 1] and the whole
weighted-scatter path runs in fp16 (the e^{-m_b} factor cancels in ctx/z).

Per-position work split: scores mult as one bulk fp16 tensor_tensor per
chunk on DVE (2x mode); score reduces split between a DVE slab tensor_reduce
and ScalarE activation+accum; weighted one-hot build via DVE
tensor_scalar_mul; context/z scatter on TensorE.

Sharding: batches LPT-balanced across 8 cores by cell count (<=128 each);
host un-permutes core outputs.
"""

import numpy as np

B, S, D, O = 1024, 200, 512, 512
NCORES = 8
P = 128                  # max batches per core
NK = D // 128            # 4 contraction chunks of 128
CS = 16                  # positions per cell
R_DVE = 9                # score reduces per chunk on DVE slab (rest ScalarE)
NEG = -1e30

_cache = {}


def _build_nc(NC):
    """Build + compile the Bass/Tile module (shared by all 8 cores)."""
    from contextlib import ExitStack

    import concourse.bass as bass
    import concourse.tile as tile
    from concourse import bacc, mybir
    from concourse.masks import make_identity

    f32 = mybir.dt.float32
    f16 = mybir.dt.float16
    AF = mybir.ActivationFunctionType
    ALU = mybir.AluOpType
    AX = mybir.AxisListType

    nc = bacc.Bacc("TRN2", target_bir_lowering=False, debug=False, num_devices=NCORES)

    x_d = nc.dram_tensor("x", [NC, P, CS * D], f16, kind="ExternalInput").ap()
    smask_d = nc.dram_tensor("smask", [NC, P, CS], f32, kind="ExternalInput").ap()
    wc_d = nc.dram_tensor("wc", [NC, P, P], f32, kind="ExternalInput").ap()
    wcf_d = nc.dram_tensor("wcf", [NC, P, P], f16, kind="ExternalInput").ap()
    wct_d = nc.dram_tensor("wct", [NC, P, P], f16, kind="ExternalInput").ap()
    lastT_d = nc.dram_tensor("lastT", [D, P], f32, kind="ExternalInput").ap()
    wqT_d = nc.dram_tensor("wqT", [D, O], f32, kind="ExternalInput").ap()
    wk_d = nc.dram_tensor("wk", [O, D], f32, kind="ExternalInput").ap()
    wvT16_d = nc.dram_tensor("wvT16", [D, O], f16, kind="ExternalInput").ap()
    bq_d = nc.dram_tensor("bq", [NK, 128, 1], f32, kind="ExternalInput").ap()
    bv16_d = nc.dram_tensor("bv16", [1, O], f16, kind="ExternalInput").ap()
    out_d = nc.dram_tensor("out", [P, O], f32, kind="ExternalOutput").ap()

    with tile.TileContext(nc) as tc:
        with ExitStack() as ctx:
            consts = ctx.enter_context(tc.tile_pool(name="consts", bufs=1))
            xc_pool = ctx.enter_context(tc.tile_pool(name="xc", bufs=4))
            t0_pool = ctx.enter_context(tc.tile_pool(name="t0", bufs=3))
            scr_pool = ctx.enter_context(tc.tile_pool(name="scr", bufs=2))
            wcw_pool = ctx.enter_context(tc.tile_pool(name="wcw", bufs=3))
            small_pool = ctx.enter_context(tc.tile_pool(name="small", bufs=4))
            ps_work = ctx.enter_context(tc.tile_pool(name="psW", bufs=2, space="PSUM"))
            ps_acc = ctx.enter_context(tc.tile_pool(name="psA", bufs=1, space="PSUM"))

            # ---------- constants ----------
            # DMA emission order = queue order: QT-chain weights first (they
            # gate the qk prologue), then wct (gathers), then the first x
            # chunks, then everything only needed from stage_c(0) onward.
            wqT_sb, wk_sb, wvT_sb, lastT_sb, bq_sb = [], [], [], [], []
            for k in range(NK):
                t = consts.tile([128, P], f32, tag=f"lastT{k}", name=f"lastT{k}")
                nc.sync.dma_start(t, lastT_d[k * 128:(k + 1) * 128, :])
                lastT_sb.append(t)
                t = consts.tile([128, O], f32, tag=f"wqT{k}", name=f"wqT{k}")
                nc.sync.dma_start(t, wqT_d[k * 128:(k + 1) * 128, :])
                wqT_sb.append(t)
                t = consts.tile([128, 1], f32, tag=f"bq{k}", name=f"bq{k}")
                nc.sync.dma_start(t, bq_d[k])
                bq_sb.append(t)
            wct_sb = []
            for c in range(NC):
                t = consts.tile([128, P], f16, tag=f"wct{c}", name=f"wct{c}")
                nc.sync.dma_start(t, wct_d[c])
                wct_sb.append(t)
            # wk only gates the qk fold (~10us in): queue it behind wct
            for k in range(NK):
                t = consts.tile([128, D], f32, tag=f"wk{k}", name=f"wk{k}")
                nc.sync.dma_start(t, wk_d[k * 128:(k + 1) * 128, :])
                wk_sb.append(t)

            xcs = {}

            def dma_x(c):
                xc = xc_pool.tile([P, CS, D], f16, tag="xc", name=f"xc{c}")
                nc.sync.dma_start(xc, x_d[c])
                xcs[c] = xc

            for c in range(min(3, NC)):
                dma_x(c)

            smask_sb = consts.tile([P, NC * CS], f32, tag="smask")
            for c in range(NC):
                nc.sync.dma_start(smask_sb[:, c * CS:(c + 1) * CS], smask_d[c])
            wc_sb, wcf_sb = [], []
            for c in range(NC):
                t = consts.tile([128, P], f32, tag=f"wc{c}", name=f"wc{c}")
                nc.sync.dma_start(t, wc_d[c])
                wc_sb.append(t)
                t = consts.tile([128, 1, P], f16, tag=f"wcf{c}", name=f"wcf{c}")
                nc.sync.dma_start(t[:, 0, :], wcf_d[c])
                wcf_sb.append(t)
            for k in range(NK):
                t = consts.tile([128, O], f16, tag=f"wvT{k}", name=f"wvT{k}")
                nc.sync.dma_start(t, wvT16_d[k * 128:(k + 1) * 128, :])
                wvT_sb.append(t)
            bv_sb = consts.tile([1, O], f16, tag="bv")
            nc.sync.dma_start(bv_sb, bv16_d)
            ones_sb = consts.tile([1, 128], f16, tag="ones")
            nc.vector.memset(ones_sb, 1.0)
            ident = consts.tile([128, 128], f32, tag="ident")
            make_identity(nc, ident)

            # ---------- QT[o,b] = Wq @ last + bq ----------
            qt_sb = []
            for om in range(NK):
                pq = ps_work.tile([128, P], f32, tag="psw", name=f"pq{om}")
                for kd in range(NK):
                    nc.tensor.matmul(
                        pq, lhsT=wqT_sb[kd][:, om * 128:(om + 1) * 128],
                        rhs=lastT_sb[kd], start=(kd == 0), stop=(kd == NK - 1))
                qt = consts.tile([128, P], f32, tag=f"qt{om}", name=f"qt{om}")
                nc.scalar.activation(qt, pq, AF.Identity, bias=bq_sb[om], scale=1.0)
                qt_sb.append(qt)

            # ---------- QK[b,d] = q^T Wk (folded query, batch-major) ----------
            pqk = ps_work.tile([P, D], f32, tag="psw", name="pqk")
            for ko in range(NK):
                nc.tensor.matmul(pqk, lhsT=qt_sb[ko], rhs=wk_sb[ko],
                                 start=(ko == 0), stop=(ko == NK - 1))
            qkb16 = consts.tile([P, D], f16, tag="qkb16")
            nc.scalar.copy(qkb16, pqk)

            # qk per cell for ALL chunks upfront: one-hot gathers on TensorE
            # (keeps the per-chunk DVE pipeline free of PE dependencies)
            qkc_sb = []
            for c in range(NC):
                pg = ps_work.tile([128, D], f32, tag="psw", name=f"pg{c}")
                nc.tensor.matmul(pg, lhsT=wct_sb[c], rhs=qkb16,
                                 start=True, stop=True)
                qkc = consts.tile([128, 1, D], f16, tag=f"qkc{c}", name=f"qkc{c}")
                nc.scalar.copy(qkc[:, 0, :], pg)
                qkc_sb.append(qkc)

            # ---------- persistent per-batch accumulators (PSUM) ----------
            ps_ctx = ps_acc.tile([P, D], f32, tag="psctx")
            ps_z = ps_acc.tile([P, 1], f32, tag="psz")

            # ---------- main loop over cell chunks (software-pipelined) ----
            # Stage A/B(c): bulk score mult + DVE slab + ScalarE reduces
            # Stage C(c): mask+exp, z, weighted one-hots, PE ctx scatter
            # C lags A/B by two chunks so no engine's in-order queue ever
            # waits on a cross-engine producer that hasn't had slack.
            OFFSET = 2
            t0s, scs = {}, {}

            def stage_ab(c):
                xc, qkc = xcs[c], qkc_sb[c]
                t0 = t0_pool.tile([P, CS, D], f16, tag="t0", name=f"t0{c}")
                nc.vector.tensor_tensor(
                    t0, xc, qkc[:, :, :].to_broadcast([P, CS, D]), ALU.mult)
                sc = small_pool.tile([P, CS], f32, tag="sc", name=f"sc{c}")
                if R_DVE > 0:
                    nc.vector.tensor_reduce(
                        sc[:, :R_DVE], t0[:, :R_DVE, :], AX.X, ALU.add)
                for si in range(R_DVE, CS):
                    scr = scr_pool.tile([P, D], f16, tag="scr",
                                        name=f"scr{c}_{si}")
                    nc.scalar.activation(scr, t0[:, si, :], AF.Copy,
                                         accum_out=sc[:, si:si + 1])
                t0s[c], scs[c] = t0, sc

            def stage_c(c):
                xc, sc = xcs[c], scs[c]
                # mask (-m_b valid / -1e30 invalid) + exp -> (0, 1]
                # mask-add on the idle Pool engine: keeps DVE's queue from
                # pausing on ScalarE's reduce completions
                nc.gpsimd.tensor_add(sc, sc, smask_sb[:, c * CS:(c + 1) * CS])
                pc = small_pool.tile([P, CS, 1], f32, tag="pc", name=f"pc{c}")
                nc.scalar.activation(pc[:, :, 0], sc, AF.Exp)

                # z per cell (a cell holds one batch): lane-reduce pc, then
                # one small f32 scatter matmul per chunk
                zcell = small_pool.tile([P, 1], f32, tag="zc", name=f"zc{c}")
                nc.vector.tensor_reduce(zcell, pc[:, :, 0], AX.X, ALU.add)
                nc.tensor.matmul(ps_z, lhsT=wc_sb[c], rhs=zcell,
                                 start=(c == 0), stop=(c == NC - 1))

                # weighted one-hots for the whole chunk in one DVE op:
                # wcw[lane, si, b] = Wc[lane, b] * pc[lane, si]
                wcw = wcw_pool.tile([128, CS, P], f16, tag="wcw",
                                    name=f"wcw{c}")
                nc.vector.tensor_tensor(
                    wcw, wcf_sb[c][:, :, :].to_broadcast([128, CS, P]),
                    pc[:, :, :].to_broadcast([P, CS, P]), ALU.mult)

                # ctx scatter on TensorE: ps_ctx += (Wc*pc_si)^T @ x_si
                for si in range(CS):
                    nc.tensor.matmul(ps_ctx, lhsT=wcw[:, si, :],
                                     rhs=xc[:, si, :],
                                     start=(c == 0 and si == 0),
                                     stop=(c == NC - 1 and si == CS - 1))

            for c in range(NC):
                stage_ab(c)
                if c + 3 < NC:
                    dma_x(c + 3)
                if c >= OFFSET:
                    stage_c(c - OFFSET)
            for c in range(max(0, NC - OFFSET), NC):
                stage_c(c)

            # ---------- normalize, project, tanh ----------
            zsb = small_pool.tile([P, 1], f32, tag="zsb")
            nc.vector.tensor_scalar_add(zsb, ps_z, 1e-30)
            rz = small_pool.tile([P, 1], f32, tag="rz")
            nc.vector.reciprocal(rz, zsb)
            ctx_fin = consts.tile([P, D], f32, tag="ctxf")
            nc.scalar.activation(ctx_fin, ps_ctx, AF.Copy, scale=rz)

            ctxT_sb = []
            for kd in range(NK):
                ptk = ps_work.tile([128, P], f32, tag="psw", name=f"ptk{kd}")
                nc.tensor.transpose(ptk, ctx_fin[:, kd * 128:(kd + 1) * 128], ident)
                t = consts.tile([128, P], f16, tag=f"ctxT{kd}", name=f"ctxT{kd}")
                nc.scalar.copy(t, ptk)
                ctxT_sb.append(t)
            pout = ps_work.tile([P, O], f32, tag="psw", name="pout")
            for kd in range(NK):
                nc.tensor.matmul(pout, lhsT=ctxT_sb[kd], rhs=wvT_sb[kd],
                                 start=(kd == 0), stop=False)
            nc.tensor.matmul(pout, lhsT=ones_sb, rhs=bv_sb, start=False, stop=True)
            outt = consts.tile([P, O], f32, tag="outt")
            nc.scalar.activation(outt, pout, AF.Tanh)
            nc.sync.dma_start(out_d, outt)

    nc.compile()
    return nc


def _host_prep(input, mask, Wq_w, Wq_b, Wk_w, Wk_b, Wv_w, Wv_b):
    """Host-side packing + sharding. Returns (per-core input maps, NC, perm)."""
    input = np.ascontiguousarray(input, dtype=np.float32)
    mask = np.asarray(mask)
    sqlen = mask.astype(np.int64).sum(axis=1)          # [B]
    last = input[np.arange(B), sqlen - 1]              # [B, D] gather
    lens = (sqlen - 1).astype(np.int64)                # valid key counts >= 1
    x16 = input.astype(np.float16)

    # Per-batch score max m_b (host-side numerics hint: keeps exp in (0,1]
    # so the weighted-scatter path can run in fp16; e^{-m_b} cancels in
    # ctx/z). Uses the same folded-query scores the device computes.
    q = last @ np.asarray(Wq_w, np.float32).T + np.asarray(Wq_b, np.float32)
    qk = q @ np.asarray(Wk_w, np.float32)              # [B, D]
    sfull = np.einsum('bd,bsd->bs', qk, input[:, :S])  # [B, S]
    valid = np.arange(S)[None, :] < lens[:, None]
    m_b = np.where(valid, sfull, -np.inf).max(axis=1).astype(np.float32)

    # LPT balance batches across cores by cell count, <=128 batches per core
    u = -(-lens // CS)                                 # cells per batch
    order = np.argsort(-u, kind="stable")
    loads = np.zeros(NCORES, np.int64)
    counts = np.zeros(NCORES, np.int64)
    core_of = np.empty(B, np.int64)
    for b in order:
        avail = np.where(counts < P)[0]
        csel = avail[np.argmin(loads[avail])]
        core_of[b] = csel
        loads[csel] += u[b]
        counts[csel] += 1
    NC = int(-(-loads.max() // P))

    wqT = np.ascontiguousarray(np.asarray(Wq_w, np.float32).T)   # [D, O]
    wk = np.ascontiguousarray(np.asarray(Wk_w, np.float32))      # [O, D]
    wvT16 = np.ascontiguousarray(np.asarray(Wv_w, np.float16).T)  # [D, O]
    bq = np.ascontiguousarray(np.asarray(Wq_b, np.float32).reshape(NK, 128, 1))
    bv16 = np.ascontiguousarray(np.asarray(Wv_b, np.float16).reshape(1, O))
    # Wk_b drops out of softmax (constant shift); Wv_b enters via ones-row matmul.

    in_maps = []
    perm = []                                          # global batch ids per core row
    for cidx in range(NCORES):
        gids = np.where(core_of == cidx)[0]
        perm.append(gids)
        nb = len(gids)

        xp = np.zeros((NC, P, CS * D), np.float16)
        smask = np.full((NC, P, CS), NEG, np.float32)
        wcf = np.zeros((NC, P, P), np.float16)

        cell = 0                                       # fill order: lane-major per chunk
        for lb, g in enumerate(gids):
            L = int(lens[g])
            s0 = 0
            while s0 < L:
                cnt = min(CS, L - s0)
                ch, lane = cell // P, cell % P
                xp[ch, lane, :cnt * D] = x16[g, s0:s0 + cnt].reshape(-1)
                smask[ch, lane, :cnt] = -m_b[g]
                wcf[ch, lane, lb] = 1.0
                cell += 1
                s0 += cnt
        wct = np.ascontiguousarray(wcf.transpose(0, 2, 1))  # [NC, b, lane]
        wc = wcf.astype(np.float32)

        lastT = np.zeros((D, P), np.float32)
        lastT[:, :nb] = last[gids].T
        in_maps.append({
            "x": xp, "smask": smask, "wc": wc, "wcf": wcf, "wct": wct,
            "lastT": np.ascontiguousarray(lastT),
            "wqT": wqT, "wk": wk, "wvT16": wvT16, "bq": bq, "bv16": bv16,
        })
    return in_maps, NC, perm


def _run(in_maps, NC, trace=False):
    from concourse.bass_utils import run_bass_kernel_spmd
    key = ("nc", NC)
    if key not in _cache:
        _cache[key] = _build_nc(NC)
    res = run_bass_kernel_spmd(_cache[key], in_maps, list(range(NCORES)),
                               trace=trace)
    return res


def _gather_out(res, perm):
    out = np.empty((B, O), np.float32)
    for cidx in range(NCORES):
        gids = perm[cidx]
        out[gids] = res.results[cidx]["out"][:len(gids)]
    return out


def kernel(input, mask, Wq_w, Wq_b, Wk_w, Wk_b, Wv_w, Wv_b):
    in_maps, NC, perm = _host_prep(input, mask, Wq_w, Wq_b,
                                   Wk_w, Wk_b, Wv_w, Wv_b)
    res = _run(in_maps, NC, trace=False)
    return _gather_out(res, perm)


# revision 55
# speedup vs baseline: 1.0639x; 1.0639x over previous
"""Trainium2 Bass kernel for AttentionLayerWithMask (ragged prefix-mask attention).

Problem: B=1024, S=200, D=O=512.
  sqlen = mask.sum(1); query = proj_q(x[b, sqlen-1]); keys/values = x[b, :sqlen-1]
  out = tanh(attn @ V)

Algebraic rewrite (exact, up to fp reassociation):
  scores[b,s] = (Wk^T q[b]) . x[b,s]  (+ const, softmax-invariant)
  out[b]      = tanh(Wv (sum_s attn[b,s] x[b,s]) + bv)

Ragged cell packing: valid key positions (avg ~102 of 200 per batch) are
packed into (lane, chunk) cells of CS positions, each cell holding tokens of
exactly one batch. A core processes 128 lanes x NC chunks (NC ~ 7 from the
actual mask, vs 8x25=200 position-slots in the dense layout). Per-chunk
one-hot routing matrices Wc tie lanes to batches: qk rows are gathered
per-cell on the TensorEngine before the score pass, and the softmax-weighted
context sum is formed by PE scatter-matmuls with pc-weighted one-hot weights
(ps_ctx += (Wc*pc_si)^T @ x_si; ps_z += (Wc*pc_si)^T @ ones), accumulated in
PSUM across all positions. The host folds per-batch score maxes m_b into the
additive mask, so pc = exp(score - m_b) lands in (0, 1] and the whole
weighted-scatter path runs in fp16 (the e^{-m_b} factor cancels in ctx/z).

Per-position work split: scores mult as one bulk fp16 tensor_tensor per
chunk on DVE (2x mode); score reduces split between a DVE slab tensor_reduce
and ScalarE activation+accum; weighted one-hot build via DVE
tensor_scalar_mul; context/z scatter on TensorE.

Sharding: batches LPT-balanced across 8 cores by cell count (<=128 each);
host un-permutes core outputs.
"""

import numpy as np

B, S, D, O = 1024, 200, 512, 512
NCORES = 8
P = 128                  # max batches per core
NK = D // 128            # 4 contraction chunks of 128
CS = 16                  # positions per cell
R_DVE = 8                # score reduces per chunk on DVE slab (rest ScalarE)
NEG = -1e30

_cache = {}


def _build_nc(NC):
    """Build + compile the Bass/Tile module (shared by all 8 cores)."""
    from contextlib import ExitStack

    import concourse.bass as bass
    import concourse.tile as tile
    from concourse import bacc, mybir
    from concourse.masks import make_identity

    f32 = mybir.dt.float32
    f16 = mybir.dt.float16
    AF = mybir.ActivationFunctionType
    ALU = mybir.AluOpType
    AX = mybir.AxisListType

    nc = bacc.Bacc("TRN2", target_bir_lowering=False, debug=False, num_devices=NCORES)

    x_d = nc.dram_tensor("x", [NC, P, CS * D], f16, kind="ExternalInput").ap()
    smask_d = nc.dram_tensor("smask", [NC, P, CS], f32, kind="ExternalInput").ap()
    wc_d = nc.dram_tensor("wc", [NC, P, P], f32, kind="ExternalInput").ap()
    wcf_d = nc.dram_tensor("wcf", [NC, P, P], f16, kind="ExternalInput").ap()
    wct_d = nc.dram_tensor("wct", [NC, P, P], f16, kind="ExternalInput").ap()
    lastT_d = nc.dram_tensor("lastT", [D, P], f32, kind="ExternalInput").ap()
    mqk_d = nc.dram_tensor("mqk", [D, D], f32, kind="ExternalInput").ap()
    c0_d = nc.dram_tensor("c0", [1, D], f32, kind="ExternalInput").ap()
    wvT16_d = nc.dram_tensor("wvT16", [D, O], f16, kind="ExternalInput").ap()
    bv16_d = nc.dram_tensor("bv16", [1, O], f16, kind="ExternalInput").ap()
    out_d = nc.dram_tensor("out", [P, O], f32, kind="ExternalOutput").ap()

    with tile.TileContext(nc) as tc:
        with ExitStack() as ctx:
            consts = ctx.enter_context(tc.tile_pool(name="consts", bufs=1))
            xc_pool = ctx.enter_context(tc.tile_pool(name="xc", bufs=4))
            t0_pool = ctx.enter_context(tc.tile_pool(name="t0", bufs=3))
            scr_pool = ctx.enter_context(tc.tile_pool(name="scr", bufs=2))
            wcw_pool = ctx.enter_context(tc.tile_pool(name="wcw", bufs=3))
            small_pool = ctx.enter_context(tc.tile_pool(name="small", bufs=4))
            ps_work = ctx.enter_context(tc.tile_pool(name="psW", bufs=2, space="PSUM"))
            ps_acc = ctx.enter_context(tc.tile_pool(name="psA", bufs=1, space="PSUM"))

            # ---------- constants ----------
            # DMA emission order = queue order: QT-chain weights first (they
            # gate the qk prologue), then wct (gathers), then the first x
            # chunks, then everything only needed from stage_c(0) onward.
            mqk_sb, wvT_sb, lastT_sb = [], [], []
            for k in range(NK):
                t = consts.tile([128, P], f32, tag=f"lastT{k}", name=f"lastT{k}")
                nc.sync.dma_start(t, lastT_d[k * 128:(k + 1) * 128, :])
                lastT_sb.append(t)
                t = consts.tile([128, D], f32, tag=f"mqk{k}", name=f"mqk{k}")
                nc.sync.dma_start(t, mqk_d[k * 128:(k + 1) * 128, :])
                mqk_sb.append(t)
            c0_sb = consts.tile([1, D], f32, tag="c0")
            nc.sync.dma_start(c0_sb, c0_d)
            ones32_sb = consts.tile([1, 128], f32, tag="ones32")
            nc.vector.memset(ones32_sb, 1.0)
            wct_sb = []
            for c in range(NC):
                t = consts.tile([128, P], f16, tag=f"wct{c}", name=f"wct{c}")
                nc.sync.dma_start(t, wct_d[c])
                wct_sb.append(t)

            xcs = {}

            def dma_x(c):
                xc = xc_pool.tile([P, CS, D], f16, tag="xc", name=f"xc{c}")
                nc.sync.dma_start(xc, x_d[c])
                xcs[c] = xc

            for c in range(min(3, NC)):
                dma_x(c)

            smask_sb = consts.tile([P, NC * CS], f32, tag="smask")
            for c in range(NC):
                nc.sync.dma_start(smask_sb[:, c * CS:(c + 1) * CS], smask_d[c])
            wc_sb, wcf_sb = [], []
            for c in range(NC):
                t = consts.tile([128, P], f32, tag=f"wc{c}", name=f"wc{c}")
                nc.sync.dma_start(t, wc_d[c])
                wc_sb.append(t)
                t = consts.tile([128, 1, P], f16, tag=f"wcf{c}", name=f"wcf{c}")
                nc.sync.dma_start(t[:, 0, :], wcf_d[c])
                wcf_sb.append(t)
            for k in range(NK):
                t = consts.tile([128, O], f16, tag=f"wvT{k}", name=f"wvT{k}")
                nc.sync.dma_start(t, wvT16_d[k * 128:(k + 1) * 128, :])
                wvT_sb.append(t)
            bv_sb = consts.tile([1, O], f16, tag="bv")
            nc.sync.dma_start(bv_sb, bv16_d)
            ones_sb = consts.tile([1, 128], f16, tag="ones")
            nc.vector.memset(ones_sb, 1.0)
            ident = consts.tile([128, 128], f32, tag="ident")
            make_identity(nc, ident)

            # ---------- QK[b,d] = last @ (Wq^T Wk) + bq Wk (host-folded) ----
            pqk = ps_work.tile([P, D], f32, tag="psw", name="pqk")
            for kd in range(NK):
                nc.tensor.matmul(pqk, lhsT=lastT_sb[kd], rhs=mqk_sb[kd],
                                 start=(kd == 0), stop=False)
            nc.tensor.matmul(pqk, lhsT=ones32_sb, rhs=c0_sb,
                             start=False, stop=True)
            qkb16 = consts.tile([P, D], f16, tag="qkb16")
            nc.scalar.copy(qkb16, pqk)

            # qk per cell for ALL chunks upfront: one-hot gathers on TensorE
            # (keeps the per-chunk DVE pipeline free of PE dependencies)
            qkc_sb = []
            for c in range(NC):
                pg = ps_work.tile([128, D], f32, tag="psw", name=f"pg{c}")
                nc.tensor.matmul(pg, lhsT=wct_sb[c], rhs=qkb16,
                                 start=True, stop=True)
                qkc = consts.tile([128, 1, D], f16, tag=f"qkc{c}", name=f"qkc{c}")
                nc.scalar.copy(qkc[:, 0, :], pg)
                qkc_sb.append(qkc)

            # ---------- persistent per-batch accumulators (PSUM) ----------
            ps_ctx = ps_acc.tile([P, D], f32, tag="psctx")
            ps_z = ps_acc.tile([P, 1], f32, tag="psz")

            # ---------- main loop over cell chunks (software-pipelined) ----
            # Stage A/B(c): bulk score mult + DVE slab + ScalarE reduces
            # Stage C(c): mask+exp, z, weighted one-hots, PE ctx scatter
            # C lags A/B by two chunks so no engine's in-order queue ever
            # waits on a cross-engine producer that hasn't had slack.
            OFFSET = 2
            t0s, scs = {}, {}

            def stage_ab(c):
                xc, qkc = xcs[c], qkc_sb[c]
                t0 = t0_pool.tile([P, CS, D], f16, tag="t0", name=f"t0{c}")
                nc.vector.tensor_tensor(
                    t0, xc, qkc[:, :, :].to_broadcast([P, CS, D]), ALU.mult)
                sc = small_pool.tile([P, CS], f32, tag="sc", name=f"sc{c}")
                if R_DVE > 0:
                    nc.vector.tensor_reduce(
                        sc[:, :R_DVE], t0[:, :R_DVE, :], AX.X, ALU.add)
                for si in range(R_DVE, CS):
                    scr = scr_pool.tile([P, D], f16, tag="scr",
                                        name=f"scr{c}_{si}")
                    nc.scalar.activation(scr, t0[:, si, :], AF.Copy,
                                         accum_out=sc[:, si:si + 1])
                t0s[c], scs[c] = t0, sc

            def stage_c(c):
                xc, sc = xcs[c], scs[c]
                # mask (-m_b valid / -1e30 invalid) + exp -> (0, 1]
                # mask-add on the idle Pool engine: keeps DVE's queue from
                # pausing on ScalarE's reduce completions
                nc.gpsimd.tensor_add(sc, sc, smask_sb[:, c * CS:(c + 1) * CS])
                pc = small_pool.tile([P, CS, 1], f32, tag="pc", name=f"pc{c}")
                nc.scalar.activation(pc[:, :, 0], sc, AF.Exp)

                # z per cell (a cell holds one batch): lane-reduce pc, then
                # one small f32 scatter matmul per chunk
                zcell = small_pool.tile([P, 1], f32, tag="zc", name=f"zc{c}")
                nc.vector.tensor_reduce(zcell, pc[:, :, 0], AX.X, ALU.add)
                nc.tensor.matmul(ps_z, lhsT=wc_sb[c], rhs=zcell,
                                 start=(c == 0), stop=(c == NC - 1))

                # weighted one-hots for the whole chunk in one DVE op:
                # wcw[lane, si, b] = Wc[lane, b] * pc[lane, si]
                wcw = wcw_pool.tile([128, CS, P], f16, tag="wcw",
                                    name=f"wcw{c}")
                nc.vector.tensor_tensor(
                    wcw, wcf_sb[c][:, :, :].to_broadcast([128, CS, P]),
                    pc[:, :, :].to_broadcast([P, CS, P]), ALU.mult)

                # ctx scatter on TensorE: ps_ctx += (Wc*pc_si)^T @ x_si
                for si in range(CS):
                    nc.tensor.matmul(ps_ctx, lhsT=wcw[:, si, :],
                                     rhs=xc[:, si, :],
                                     start=(c == 0 and si == 0),
                                     stop=(c == NC - 1 and si == CS - 1))

            for c in range(NC):
                stage_ab(c)
                if c + 3 < NC:
                    dma_x(c + 3)
                if c >= OFFSET:
                    stage_c(c - OFFSET)
            for c in range(max(0, NC - OFFSET), NC):
                stage_c(c)

            # ---------- normalize, project, tanh ----------
            zsb = small_pool.tile([P, 1], f32, tag="zsb")
            nc.vector.tensor_scalar_add(zsb, ps_z, 1e-30)
            rz = small_pool.tile([P, 1], f32, tag="rz")
            nc.vector.reciprocal(rz, zsb)
            ctx_fin = consts.tile([P, D], f32, tag="ctxf")
            nc.scalar.activation(ctx_fin, ps_ctx, AF.Copy, scale=rz)

            ctxT_sb = []
            for kd in range(NK):
                ptk = ps_work.tile([128, P], f32, tag="psw", name=f"ptk{kd}")
                nc.tensor.transpose(ptk, ctx_fin[:, kd * 128:(kd + 1) * 128], ident)
                t = consts.tile([128, P], f16, tag=f"ctxT{kd}", name=f"ctxT{kd}")
                nc.scalar.copy(t, ptk)
                ctxT_sb.append(t)
            pout = ps_work.tile([P, O], f32, tag="psw", name="pout")
            for kd in range(NK):
                nc.tensor.matmul(pout, lhsT=ctxT_sb[kd], rhs=wvT_sb[kd],
                                 start=(kd == 0), stop=False)
            nc.tensor.matmul(pout, lhsT=ones_sb, rhs=bv_sb, start=False, stop=True)
            outt = consts.tile([P, O], f32, tag="outt")
            nc.scalar.activation(outt, pout, AF.Tanh)
            nc.sync.dma_start(out_d, outt)

    nc.compile()
    return nc


def _host_prep(input, mask, Wq_w, Wq_b, Wk_w, Wk_b, Wv_w, Wv_b):
    """Host-side packing + sharding. Returns (per-core input maps, NC, perm)."""
    input = np.ascontiguousarray(input, dtype=np.float32)
    mask = np.asarray(mask)
    sqlen = mask.astype(np.int64).sum(axis=1)          # [B]
    last = input[np.arange(B), sqlen - 1]              # [B, D] gather
    lens = (sqlen - 1).astype(np.int64)                # valid key counts >= 1
    x16 = input.astype(np.float16)

    # Per-batch score max m_b (host-side numerics hint: keeps exp in (0,1]
    # so the weighted-scatter path can run in fp16; e^{-m_b} cancels in
    # ctx/z). Uses the same folded-query scores the device computes.
    q = last @ np.asarray(Wq_w, np.float32).T + np.asarray(Wq_b, np.float32)
    qk = q @ np.asarray(Wk_w, np.float32)              # [B, D]
    sfull = np.einsum('bd,bsd->bs', qk, input[:, :S])  # [B, S]
    valid = np.arange(S)[None, :] < lens[:, None]
    m_b = np.where(valid, sfull, -np.inf).max(axis=1).astype(np.float32)

    # LPT balance batches across cores by cell count, <=128 batches per core
    u = -(-lens // CS)                                 # cells per batch
    order = np.argsort(-u, kind="stable")
    loads = np.zeros(NCORES, np.int64)
    counts = np.zeros(NCORES, np.int64)
    core_of = np.empty(B, np.int64)
    for b in order:
        avail = np.where(counts < P)[0]
        csel = avail[np.argmin(loads[avail])]
        core_of[b] = csel
        loads[csel] += u[b]
        counts[csel] += 1
    NC = int(-(-loads.max() // P))

    wq32 = np.asarray(Wq_w, np.float32)
    wk32 = np.asarray(Wk_w, np.float32)
    mqk = np.ascontiguousarray(wq32.T @ wk32)                     # [D, D]
    c0 = np.ascontiguousarray(
        (np.asarray(Wq_b, np.float32) @ wk32).reshape(1, D))
    wvT16 = np.ascontiguousarray(np.asarray(Wv_w, np.float16).T)  # [D, O]
    bv16 = np.ascontiguousarray(np.asarray(Wv_b, np.float16).reshape(1, O))
    # Wk_b drops out of softmax (constant shift); Wv_b enters via ones-row matmul.

    in_maps = []
    perm = []                                          # global batch ids per core row
    for cidx in range(NCORES):
        gids = np.where(core_of == cidx)[0]
        perm.append(gids)
        nb = len(gids)

        xp = np.zeros((NC, P, CS * D), np.float16)
        smask = np.full((NC, P, CS), NEG, np.float32)
        wcf = np.zeros((NC, P, P), np.float16)

        cell = 0                                       # fill order: lane-major per chunk
        for lb, g in enumerate(gids):
            L = int(lens[g])
            s0 = 0
            while s0 < L:
                cnt = min(CS, L - s0)
                ch, lane = cell // P, cell % P
                xp[ch, lane, :cnt * D] = x16[g, s0:s0 + cnt].reshape(-1)
                smask[ch, lane, :cnt] = -m_b[g]
                wcf[ch, lane, lb] = 1.0
                cell += 1
                s0 += cnt
        wct = np.ascontiguousarray(wcf.transpose(0, 2, 1))  # [NC, b, lane]
        wc = wcf.astype(np.float32)

        lastT = np.zeros((D, P), np.float32)
        lastT[:, :nb] = last[gids].T
        in_maps.append({
            "x": xp, "smask": smask, "wc": wc, "wcf": wcf, "wct": wct,
            "lastT": np.ascontiguousarray(lastT),
            "mqk": mqk, "c0": c0, "wvT16": wvT16, "bv16": bv16,
        })
    return in_maps, NC, perm


def _run(in_maps, NC, trace=False):
    from concourse.bass_utils import run_bass_kernel_spmd
    key = ("nc", NC)
    if key not in _cache:
        _cache[key] = _build_nc(NC)
    res = run_bass_kernel_spmd(_cache[key], in_maps, list(range(NCORES)),
                               trace=trace)
    return res


def _gather_out(res, perm):
    out = np.empty((B, O), np.float32)
    for cidx in range(NCORES):
        gids = perm[cidx]
        out[gids] = res.results[cidx]["out"][:len(gids)]
    return out


def kernel(input, mask, Wq_w, Wq_b, Wk_w, Wk_b, Wv_w, Wv_b):
    in_maps, NC, perm = _host_prep(input, mask, Wq_w, Wq_b,
                                   Wk_w, Wk_b, Wv_w, Wv_b)
    res = _run(in_maps, NC, trace=False)
    return _gather_out(res, perm)
